# revision 64
# baseline (speedup 1.0000x reference)
"""Trainium2 Bass kernel for a GQA attention block (dense_transformer).

Reference computation (per core c of 8, tensor-parallel over heads):
  q = x @ wq[:, 256c:256c+256]   -> 2 query heads of dim 128
  k = x @ wk[:, 128g:128g+128]   -> 1 kv head (g = c//2, shared by 2 cores)
  v = x @ wv[:, 128g:128g+128]
  RoPE on q, k; causal softmax attention; o = attn @ v
  out_partial = o @ wo[256c:256c+256, :]     (full [4096, 2048] partial sum)
Host sums the 8 partials.

Device layout: everything transposed ([dim, seq]) so matmuls need no
on-chip transposes of activations:
  QT/KT:  [128 d, 4096 s]  (from projections; RoPE applied on evacuation)
  scores^T[k, q] = KT_blk.T @ QT  (lhsT=KT block, rhs=QT columns)
  P^T = exp(scores^T) (ACT, scale=1/sqrt(128)); causal via 0/1 bf16 mask mul
  O^T += V_blk.T @ P^T            (lhsT=V block [s,d], rhs=P^T)
  row sums via ones-matmul into psum; normalization via K=1 broadcast matmul
  out[s, dm] = O^T_blk.T @ wo     (lhsT=O^T block, rhs=wo rows)

All matmuls are bf16 (fp8 q/k was tried and rejected: 4.5e-2 rel err vs the
2e-2 gate — with random-init weights softmax does NOT wash out logit noise).
Host pre-arranges x^T (and weights) so every DMA reads contiguous DRAM.
"""

import os
import numpy as np
import ml_dtypes

S = 4096
DM = 2048
HD = 128
NCORES = 8
QSUP = 512          # query supertile (free dim of scores^T psum)
NT = S // QSUP      # 8
NKB = S // 128      # 32 key blocks
SCALE = float(1.0 / np.sqrt(HD))
THETA = 10000.0

_CACHE = {}


def _tctile(tc, shape, dtype, name):
    return tc.nc.alloc_sbuf_tensor(name, list(shape), dtype).ap()



DEFAULT_CFG = dict(
    sums_on_dve=True,    # accumulate softmax row-sums on DVE (PE matmul-sums
                         # measured much slower on HW)
    out_f16=True,        # fp16 partial output (host sums in fp32)
    scps_bufs=3,         # psum bufs for scores (+shared r/out-proj/vt tiles)
    prps_bufs=2,         # psum bufs for projection accumulators
    pt_bufs=4,           # sbuf bufs for exp(P^T) tiles
    evac_alt=True,       # alternate out-proj psum evacuation DVE/ACT
    sums_gpsimd=False,   # offload head-1 row-sum adds to the idle Pool engine
    oacc_bufs=1,         # psum bufs per oacc head (2 overlaps t and t+1)
    rope_evac_dve=True,  # psum->sbuf rope copy on DVE instead of ACT
    diag_skip=True,      # skip all-zero columns [0,128j) of diagonal blocks
    r_dual_psum=False,   # ILLEGAL on HW: TT cannot read 2 PSUM operands
    xt_split=4,          # split the per-chunk x^T load into N DMAs
    sbp_bufs=2,          # bufs for the xt/cos/sin input pool
    fp8_qk=False,        # fp8 q/k proj: rel err 4.5e-2 > 2e-2 gate (dead)
    w_split=4,           # split weight DMAs into N kc-chunks (early start)
    out_pair=2,          # output subtiles per store DMA
    out_psum_dma=False,  # (unsupported: DMA cannot read PSUM)
    sums_split=False,    # even/odd kb sum accumulators (halves serial chain)
    evac_dve_tail=8,     # supertiles >= this evacuate out-proj on DVE only
    oproj_defer=0,       # defer first N supertiles' out-proj to the end
    skip_out_store=False,   # timing experiment: drop output stores
    skip_x_load=False,      # timing experiment: reread chunk 0 as all chunks
    fuse_attnv=False,    # ILLEGAL: fused psum out would span 2 banks
    x_contig=True,       # host pre-chunks xT so each chunk loads contiguously
    w_contig=True,       # host pre-transposes weights for contiguous loads
    ot_raw=True,         # evacuate oacc unnormalized (frees psum bank before
                         # the recip chain), normalize OT in place on DVE
    r_evac_dve=False,    # r_ps psum->sbuf copy on ACT (HW A/B: ~5us better
                         # than DVE, which is loaded by the raw OT copies)
    exp_pair_h=False,    # one exp instruction covering both heads' scores
                         # (2-bank psum tile; halves ACT instruction count)
    swap_act_dge=False,  # rope-swap DMAs on the ACT DGE ring: HW A/B was
                         # noise-contradictory; keep the validated SP ring
)


def _build_nc(loop_iters=1, cfg=None):
    import contextlib
    import concourse.mybir as mybir
    import concourse.tile as tile
    from concourse import bacc
    from concourse.masks import make_identity

    cfg = {**DEFAULT_CFG, **(cfg or {})}
    dt = mybir.dt
    f32 = dt.float32
    bf16 = dt.bfloat16
    f8 = dt.float8e4
    out_dt = f32 if cfg["out_psum_dma"] else (
        dt.float16 if cfg["out_f16"] else f32)

    nc = bacc.Bacc("TRN2", target_bir_lowering=False, debug=False, num_devices=NCORES)

    # x_contig: rows = sc*128 + p, cols = kc*512 + s (chunk-contiguous)
    xT_shape = [8 * 128, 16 * 512] if cfg["x_contig"] else [DM, S]
    xT_d = nc.dram_tensor("xT", xT_shape, bf16, kind="ExternalInput")
    wv_shape = [128, 16 * 128] if cfg["w_contig"] else [DM, 128]
    wv_d = nc.dram_tensor("wv", wv_shape, bf16, kind="ExternalInput")
    wo_d = nc.dram_tensor("wo", [256, DM], bf16, kind="ExternalInput")
    csT_d = nc.dram_tensor("csT", [HD, 2 * S], bf16, kind="ExternalInput")
    maskB_d = nc.dram_tensor("maskB", [128, 4 * QSUP], bf16, kind="ExternalInput")
    out_d = nc.dram_tensor("out", [S, DM], out_dt, kind="ExternalOutput")
    wq_shape = [128, 16 * 256] if cfg["w_contig"] else [DM, 256]
    wk_shape = [128, 16 * 128] if cfg["w_contig"] else [DM, 128]
    if cfg["fp8_qk"]:
        xT8_d = nc.dram_tensor("xT8", [DM, S], f8, kind="ExternalInput")
        wq_d = nc.dram_tensor("wq", wq_shape, f8, kind="ExternalInput")
        wk_d = nc.dram_tensor("wk", wk_shape, f8, kind="ExternalInput")
    else:
        xT8_d = None
        wq_d = nc.dram_tensor("wq", wq_shape, bf16, kind="ExternalInput")
        wk_d = nc.dram_tensor("wk", wk_shape, bf16, kind="ExternalInput")


    qk_dt = f8 if cfg["fp8_qk"] else bf16

    with tile.TileContext(nc) as tc:
        # ---- persistent sbuf tensors ----
        QT0 = _tctile(tc, [128, S], bf16, name="QT0")
        QT1 = _tctile(tc, [128, S], bf16, name="QT1")
        KT = _tctile(tc, [128, S], bf16, name="KT")
        VT = _tctile(tc, [128, S], bf16, name="VT")    # [d, s] pre-transpose
        V = _tctile(tc, [128, S], bf16, name="V")      # [s, d] blocks at cols 128*kb
        OT0 = _tctile(tc, [128, S], bf16, name="OT0")
        OT1 = _tctile(tc, [128, S], bf16, name="OT1")
        wq_sb = _tctile(tc, [128, 16 * 256], qk_dt, name="wq_sb")
        wk_sb = _tctile(tc, [128, 16 * 128], qk_dt, name="wk_sb")
        wv_sb = _tctile(tc, [128, 16 * 128], bf16, name="wv_sb")
        wo0_sb = _tctile(tc, [128, DM], bf16, name="wo0_sb")
        wo1_sb = _tctile(tc, [128, DM], bf16, name="wo1_sb")
        maskB = _tctile(tc, [128, 4 * QSUP], bf16, name="maskB_sb")
        ident = _tctile(tc, [128, 128], bf16, name="ident")
        onescols = _tctile(tc, [128, 3], bf16, name="onescols")
        ones1 = _tctile(tc, [33, 128], dt.float16, name="ones1")

        # constants
        make_identity(nc, ident[:, :])
        nc.gpsimd.memset(onescols[:, :], 0.0)
        nc.gpsimd.memset(onescols[:, 0:1], 1.0)
        nc.gpsimd.memset(onescols[:, 2:3], 1.0)
        nc.gpsimd.memset(ones1[0:1, :], 1.0)
        nc.gpsimd.memset(ones1[32:33, :], 1.0)

        env = dict(locals())
        loop_ctx = (tc.For_i(0, loop_iters, 1) if loop_iters > 1
                    else contextlib.nullcontext())
        with loop_ctx:
            _emit_body(nc, tc, mybir, cfg, env)

    nc.compile()
    return nc


def _emit_body(nc, tc, mybir, cfg, env):
    dt = mybir.dt
    f32 = dt.float32
    bf16 = dt.bfloat16
    f16 = dt.float16
    f8 = dt.float8e4
    out_dt = f32 if cfg["out_psum_dma"] else (f16 if cfg["out_f16"] else f32)
    AF = mybir.ActivationFunctionType
    PM = mybir.MatmulPerfMode
    xT = env["xT_d"].ap()
    out = env["out_d"].ap()
    csT_d = env["csT_d"]
    QT0, QT1, KT, VT, V, OT0, OT1 = (env[k] for k in
                                     ("QT0", "QT1", "KT", "VT", "V", "OT0", "OT1"))
    wq_sb, wk_sb, wv_sb, wo0_sb, wo1_sb = (env[k] for k in
                                           ("wq_sb", "wk_sb", "wv_sb",
                                            "wo0_sb", "wo1_sb"))
    maskB, ident, onescols, ones1 = (env[k] for k in
                                     ("maskB", "ident", "onescols", "ones1"))
    QTs = [QT0, QT1]
    OTs = [OT0, OT1]
    fp8_qk = cfg["fp8_qk"]

    def x_src(sc):
        """[128, 16, 512] source AP for chunk sc of x^T."""
        if cfg["x_contig"]:
            return (xT[128 * sc:128 * sc + 128, :]
                    .rearrange("p (kc s) -> p kc s", s=512))
        cs = slice(512 * sc, 512 * sc + 512)
        return xT.rearrange("(kc p) s -> p kc s", p=128)[:, :, cs]

    with tc.tile_pool(name="sbp", bufs=cfg["sbp_bufs"]) as sbp, \
         tc.tile_pool(name="prps", bufs=cfg["prps_bufs"], space="PSUM") as prps, \
         tc.tile_pool(name="scps", bufs=cfg["scps_bufs"], space="PSUM") as scps, \
         tc.tile_pool(name="oaps", bufs=cfg["oacc_bufs"], space="PSUM") as oaps, \
         tc.tile_pool(name="smps", bufs=1, space="PSUM") as smps, \
         tc.tile_pool(name="rop", bufs=3) as rop, \
         tc.tile_pool(name="ptpool", bufs=cfg["pt_bufs"]) as ptpool, \
         tc.tile_pool(name="nrm", bufs=2) as nrm, \
         tc.tile_pool(name="outsb", bufs=3) as outsb:

        def wps(shape, dtype, name):
            """Working psum tile: from prps in exp_pair_h mode (the scores
            ring then holds only 2-bank sc2 tiles), else from scps."""
            if cfg["exp_pair_h"]:
                return prps.tile(shape, dtype, tag="proj", name=name)
            return scps.tile(shape, dtype, tag="sc", name=name)

        def load_chunk(sc):
            """Issue input DMAs for s-chunk sc (512 wide); return tiles.

            fp8 x loads first (q/k proj consume it first); few big DMAs —
            each DMA instruction holds the shared HWDGE ring ~625ns, so
            instruction count dominates ring occupancy, not bytes.
            """
            if cfg["skip_x_load"]:
                sc = 0
            cs = slice(512 * sc, 512 * sc + 512)
            nsp = cfg["xt_split"]
            gk = 16 // nsp
            xt8 = None
            if fp8_qk:
                xt8 = sbp.tile([128, 16 * 512], f8, tag="xt8", name=f"xt8_{sc}")
                xt83 = xt8.rearrange("p (kc s) -> p kc s", s=512)
                src83 = (env["xT8_d"].ap()
                         .rearrange("(kc p) s -> p kc s", p=128)[:, :, cs])
                for g in range(nsp):
                    nc.sync.dma_start(xt83[:, gk * g:gk * (g + 1), :],
                                      src83[:, gk * g:gk * (g + 1), :])
            cs_t = sbp.tile([128, 1024], bf16, tag="cs", name=f"cs_{sc}")
            nc.sync.dma_start(
                cs_t.rearrange("p (i s) -> p i s", s=512),
                csT_d.ap().rearrange("p (i s) -> p i s", s=S)[:, :, cs])
            xt = sbp.tile([128, 16 * 512], bf16, tag="xt", name=f"xt_{sc}")
            xt3 = xt.rearrange("p (kc s) -> p kc s", s=512)
            src3 = x_src(sc)
            for g in range(nsp):
                nc.sync.dma_start(xt3[:, gk * g:gk * (g + 1), :],
                                  src3[:, gk * g:gk * (g + 1), :])
            return xt, xt8, cs_t[:, 0:512], cs_t[:, 512:1024]

        def load_preamble():
            """Weights + chunk-0 inputs, interleaved so the first q-proj
            matmul (wq kc-pair 0 + xt8 kc 0..1) unblocks ASAP."""
            nw = cfg["w_split"]
            gk = 16 // nw
            wq3 = wq_sb.rearrange("p (kc c) -> p kc c", c=256)
            wk3 = wk_sb.rearrange("p (kc c) -> p kc c", c=128)
            wv3 = wv_sb.rearrange("p (kc c) -> p kc c", c=128)
            if cfg["w_contig"]:
                wqs = env["wq_d"].ap().rearrange("p (kc c) -> p kc c", c=256)
                wks = env["wk_d"].ap().rearrange("p (kc c) -> p kc c", c=128)
                wvs = env["wv_d"].ap().rearrange("p (kc c) -> p kc c", c=128)
            else:
                wqs = env["wq_d"].ap().rearrange("(kc p) c -> p kc c", p=128)
                wks = env["wk_d"].ap().rearrange("(kc p) c -> p kc c", p=128)
                wvs = env["wv_d"].ap().rearrange("(kc p) c -> p kc c", p=128)
            cs = slice(0, 512)
            nsp = cfg["xt_split"]
            gx = 16 // nsp
            xt8 = None
            if fp8_qk:
                xt8 = sbp.tile([128, 16 * 512], f8, tag="xt8", name="xt8_0")
                xt83 = xt8.rearrange("p (kc s) -> p kc s", s=512)
                src83 = (env["xT8_d"].ap()
                         .rearrange("(kc p) s -> p kc s", p=128)[:, :, cs])
            xt = sbp.tile([128, 16 * 512], bf16, tag="xt", name="xt_0")
            xt3 = xt.rearrange("p (kc s) -> p kc s", s=512)
            src3 = x_src(0)
            # q/k weights + their x operand interleaved by kc quarter
            for g in range(max(nw, nsp)):
                if g < nw:
                    ks = slice(gk * g, gk * (g + 1))
                    nc.sync.dma_start(wq3[:, ks, :], wqs[:, ks, :])
                    nc.sync.dma_start(wk3[:, ks, :], wks[:, ks, :])
                if fp8_qk and g < nsp:
                    xs = slice(gx * g, gx * (g + 1))
                    nc.sync.dma_start(xt83[:, xs, :], src83[:, xs, :])
                if not fp8_qk and g < nsp:
                    xs = slice(gx * g, gx * (g + 1))
                    nc.sync.dma_start(xt3[:, xs, :], src3[:, xs, :])
            cs_t = sbp.tile([128, 1024], bf16, tag="cs", name="cs_0")
            nc.sync.dma_start(
                cs_t.rearrange("p (i s) -> p i s", s=512),
                csT_d.ap().rearrange("p (i s) -> p i s", s=S)[:, :, cs])
            # v weights (+ bf16 x when the q/k path is fp8)
            for g in range(max(nw, nsp)):
                if g < nw:
                    ks = slice(gk * g, gk * (g + 1))
                    nc.sync.dma_start(wv3[:, ks, :], wvs[:, ks, :])
                if fp8_qk and g < nsp:
                    xs = slice(gx * g, gx * (g + 1))
                    nc.sync.dma_start(xt3[:, xs, :], src3[:, xs, :])
            nc.sync.dma_start(maskB[:, :], env["maskB_d"].ap()[:, :])
            nc.sync.dma_start(wo0_sb[:, :], env["wo_d"].ap()[0:128, :])
            nc.sync.dma_start(wo1_sb[:, :], env["wo_d"].ap()[128:256, :])
            return xt, xt8, cs_t[:, 0:512], cs_t[:, 512:1024]

        def proj_chunk(sc, ins):
            """Projections + RoPE + V transpose for s-chunk sc (512 wide)."""
            cs = slice(512 * sc, 512 * sc + 512)
            xt, xt8, cos_t, sin_t = ins
            xt83 = (xt8.rearrange("p (kc s) -> p kc s", s=512)
                    if fp8_qk else None)

            def proj(w_sb, wstride, hofs, name):
                ps = prps.tile([128, 512], f32, tag="proj", name=name)
                for kc in range(16):
                    nc.tensor.matmul(
                        ps[:, :],
                        w_sb[:, wstride * kc + hofs:wstride * kc + hofs + 128],
                        xt[:, 512 * kc:512 * kc + 512],
                        start=(kc == 0), stop=(kc == 15))
                return ps

            def proj8(w_sb, wstride, hofs, name):
                ps = prps.tile([128, 512], f32, tag="proj", name=name)
                w3 = w_sb.rearrange("p (kc c) -> p kc c", c=wstride)
                for kp in range(8):
                    nc.tensor.matmul(
                        ps[:, :],
                        w3[:, 2 * kp:2 * kp + 2, hofs:hofs + 128],
                        xt83[:, 2 * kp:2 * kp + 2, :],
                        start=(kp == 0), stop=(kp == 7),
                        perf_mode=PM.DoubleRow)
                return ps

            pj = proj8 if fp8_qk else proj

            # shared raw/swap tiles for q0|q1|k: one swap DMA pair per chunk
            qraw3 = rop.tile([128, 1536], bf16, tag="qraw", name=f"qraw_{sc}")
            qsw3 = rop.tile([128, 1536], bf16, tag="qsw", name=f"qsw_{sc}")
            pss = [pj(wq_sb, 256, 0, f"psq0_{sc}"),
                   pj(wq_sb, 256, 128, f"psq1_{sc}"),
                   pj(wk_sb, 128, 0, f"psk_{sc}")]
            for h, ps in enumerate(pss):
                hs = slice(512 * h, 512 * h + 512)
                if cfg["rope_evac_dve"]:
                    nc.vector.tensor_copy(qraw3[:, hs], ps[:, :])
                else:
                    nc.scalar.copy(qraw3[:, hs], ps[:, :])
            dge = nc.scalar if cfg["swap_act_dge"] else nc.sync
            dge.dma_start(qsw3[0:64, :], qraw3[64:128, :])
            dge.dma_start(qsw3[64:128, :], qraw3[0:64, :])
            for h, dst in enumerate([QT0, QT1, KT]):
                hs = slice(512 * h, 512 * h + 512)
                m1 = rop.tile([128, 512], bf16, tag="m1", name=f"m1_{sc}_{h}")
                nc.vector.tensor_mul(m1[:, :], qraw3[:, hs], cos_t[:, :])
                nc.vector.tensor_mul(qsw3[:, hs], qsw3[:, hs], sin_t[:, :])
                nc.vector.tensor_add(dst[:, cs], m1[:, :], qsw3[:, hs])
            psv = proj(wv_sb, 128, 0, f"psv_{sc}")
            nc.vector.tensor_copy(VT[:, cs], psv[:, :])
            for kb in range(4 * sc, 4 * sc + 4):
                bs = slice(128 * kb, 128 * kb + 128)
                tp = wps([128, 128], bf16, name=f"vtp_{kb}")
                nc.tensor.transpose(tp[:, :], VT[:, bs], ident[:, :])
                nc.vector.tensor_copy(V[:, bs], tp[:, :])

        def attn_supertile(t):
            qs = slice(QSUP * t, QSUP * t + QSUP)
            nkb = 4 * t + 4
            ep = cfg["exp_pair_h"]
            fuse = cfg["fuse_attnv"]
            if fuse:
                oacc2 = oaps.tile([128, 2 * QSUP], f32, tag="oacc2",
                                  name=f"oacc2_{t}")
                oacc = [oacc2[:, 0:QSUP], oacc2[:, QSUP:2 * QSUP]]
            else:
                oacc = [oaps.tile([128, QSUP], f32, tag="oacc0",
                                  name=f"oacc0_{t}"),
                        oaps.tile([128, QSUP], f32, tag="oacc1",
                                  name=f"oacc1_{t}")]
            # rows {0, 32} of one psum bank (32-aligned for later matmul rhs)
            # with exp_pair_h the sums tile borrows a scores-ring slot at the
            # END of the supertile (early allocation would deadlock the ring)
            sums = (None if ep else
                    smps.tile([33, QSUP], f32, tag="sums", name=f"sums_{t}"))
            nsac = 2 if (cfg["sums_split"] and nkb > 4) else 1
            if cfg["sums_on_dve"]:
                sacc = [[nrm.tile([128, QSUP], bf16, tag=f"sacc{h}_{p}",
                                  name=f"sacc{h}_{p}_{t}")
                         for p in range(nsac)] for h in range(2)]
                sfirst = [[True] * nsac for _ in range(2)]
            prev_pt = [None, None]
            for kb in range(nkb):
                bs = slice(128 * kb, 128 * kb + 128)
                j = kb - 4 * t
                # columns [0, z) of a diagonal block are entirely masked
                z = 128 * j if (cfg["diag_skip"] and j > 0) else 0
                zq = slice(QSUP * t + z, QSUP * t + QSUP)
                pt2 = (ptpool.tile([128, 2 * QSUP], bf16, tag="pt",
                                   name=f"pt2_{t}_{kb}") if (fuse or ep)
                       else None)
                if ep:
                    sc2 = scps.tile([128, 2 * QSUP], f32, tag="sc",
                                    name=f"sc2_{t}_{kb}")
                    for h in range(2):
                        nc.tensor.matmul(
                            sc2[:, QSUP * h + z:QSUP * (h + 1)],
                            KT[:, bs], QTs[h][:, zq], start=True, stop=True)
                    if z:
                        nc.scalar.activation(
                            pt2.rearrange("p (h s) -> p h s", s=QSUP)[:, :, z:],
                            sc2.rearrange("p (h s) -> p h s", s=QSUP)[:, :, z:],
                            AF.Exp, scale=SCALE)
                    else:
                        nc.scalar.activation(pt2[:, :], sc2[:, :], AF.Exp,
                                             scale=SCALE)
                for h in range(2):
                    if ep:
                        pt = pt2[:, QSUP * h:QSUP * h + QSUP]
                    else:
                        sc_ps = scps.tile([128, QSUP], f32, tag="sc",
                                          name=f"sc_{t}_{kb}_{h}")
                        nc.tensor.matmul(sc_ps[:, z:], KT[:, bs],
                                         QTs[h][:, zq],
                                         start=True, stop=True)
                        pt = (pt2[:, QSUP * h:QSUP * h + QSUP] if fuse else
                              ptpool.tile([128, QSUP], bf16, tag="pt",
                                          name=f"pt_{t}_{kb}_{h}"))
                        nc.scalar.activation(pt[:, z:], sc_ps[:, z:], AF.Exp,
                                             scale=SCALE)
                    if 0 <= j:
                        w = 128 * (j + 1)   # cols >= w are fully unmasked
                        nc.vector.tensor_mul(
                            pt[:, z:w], pt[:, z:w],
                            maskB[:, QSUP * j + z:QSUP * j + w])
                    if not fuse:
                        nc.tensor.matmul(oacc[h][:, z:], V[:, bs], pt[:, z:],
                                         start=(kb == 0), stop=(kb == nkb - 1))
                    if cfg["sums_on_dve"]:
                        eng = (nc.gpsimd if (cfg["sums_gpsimd"] and h == 1)
                               else nc.vector)
                        p = kb % nsac
                        sa = sacc[h][p]
                        if sfirst[h][p]:
                            sfirst[h][p] = False
                            eng.tensor_copy(sa[:, z:], pt[:, z:])
                            if z:
                                nc.gpsimd.memset(sa[:, 0:z], 0.0)
                        else:
                            eng.tensor_add(sa[:, z:], sa[:, z:], pt[:, z:])
                    else:
                        nc.tensor.matmul(sums[32 * h:32 * h + 1, :],
                                         onescols[:, 0:1], pt[:, :],
                                         start=(kb == 0),
                                         stop=(kb == nkb - 1))
                if fuse:
                    rhs3 = pt2.rearrange("p (h s) -> p h s", s=QSUP)[:, :, z:]
                    out3 = (oacc2.rearrange("p (h s) -> p h s", s=QSUP)
                            [:, :, z:])
                    nc.tensor.matmul(out3, V[:, bs], rhs3,
                                     start=(kb == 0), stop=(kb == nkb - 1))
            if ep:
                sums = scps.tile([33, QSUP], f32, tag="sc", name=f"sums_{t}")
            if cfg["sums_on_dve"]:
                for h in range(2):
                    for p in range(nsac):
                        nc.tensor.matmul(sums[32 * h:32 * h + 1, :],
                                         onescols[:, 0:1], sacc[h][p][:, :],
                                         start=(p == 0), stop=(p == nsac - 1))
            # free the oacc psum bank ASAP: raw copy now, normalize in place
            if cfg["ot_raw"]:
                for h in range(2):
                    nc.vector.tensor_copy(OTs[h][:, qs], oacc[h][:, :])
            # normalize: rs = 1/sums (fp16), broadcast via K=1 fp16 matmul
            rs = nrm.tile([33, QSUP], f16, tag="rs", name=f"rs_{t}")
            with nc.allow_low_precision(reason="fp16 softmax normalizer"):
                nc.vector.reciprocal(rs[0:1, :], sums[0:1, :])
                nc.vector.reciprocal(rs[32:33, :], sums[32:33, :])
            for h in range(2):
                r_ps = wps([128, QSUP], f32, name=f"rps_{t}_{h}")
                nc.tensor.matmul(r_ps[:, :], ones1[32 * h:32 * h + 1, :],
                                 rs[32 * h:32 * h + 1, :],
                                 start=True, stop=True)
                r_sb = nrm.tile([128, QSUP], f16 if cfg["ot_raw"] else f32,
                                tag="rsb", name=f"rsb_{t}_{h}")
                if cfg["r_evac_dve"]:
                    nc.vector.tensor_copy(r_sb[:, :], r_ps[:, :])
                else:
                    nc.scalar.copy(r_sb[:, :], r_ps[:, :])
                if cfg["ot_raw"]:
                    nc.vector.tensor_mul(OTs[h][:, qs], OTs[h][:, qs],
                                         r_sb[:, :])
                else:
                    nc.vector.tensor_mul(OTs[h][:, qs], oacc[h][:, :],
                                         r_sb[:, :])
            if t < cfg["oproj_defer"]:
                return      # emitted at the end: PE filler for the exp tail
            out_proj(t)

        def out_proj(t):
            # out-projection for the 4 s-subtiles of supertile t;
            # out_pair subtiles share one sbuf tile -> one store DMA
            npair = 1 if t == NT - 1 else cfg["out_pair"]
            use_act = cfg["evac_alt"] and t < cfg["evac_dve_tail"]
            for u in range(4 // npair):
                ob = outsb.tile([128, npair * DM], out_dt, tag="ob",
                                name=f"ob_{t}_{u}")
                for two in range(npair):
                    st = 4 * t + npair * u + two
                    ss = slice(128 * st, 128 * st + 128)
                    for ncol in range(4):
                        o_ps = wps([128, 512], f32, name=f"op_{st}_{ncol}")
                        nc.tensor.matmul(o_ps[:, :], OT0[:, ss],
                                         wo0_sb[:, 512 * ncol:512 * ncol + 512],
                                         start=True, stop=False)
                        nc.tensor.matmul(o_ps[:, :], OT1[:, ss],
                                         wo1_sb[:, 512 * ncol:512 * ncol + 512],
                                         start=False, stop=True)
                        oc = slice(DM * two + 512 * ncol,
                                   DM * two + 512 * ncol + 512)
                        if use_act and (ncol % 2 == 1):
                            nc.scalar.copy(ob[:, oc], o_ps[:, :])
                        else:
                            nc.vector.tensor_copy(ob[:, oc], o_ps[:, :])
                rb = 128 * (4 * t + npair * u)
                if cfg["skip_out_store"]:
                    pass
                elif npair == 1:
                    nc.sync.dma_start(out[rb:rb + 128, :], ob[:, :])
                else:
                    nc.sync.dma_start(
                        out[rb:rb + 128 * npair, :]
                        .rearrange("(two p) c -> p two c", p=128),
                        ob.rearrange("p (two c) -> p two c", c=DM))

        pre = load_preamble()
        for sc in range(8):
            ins = pre if sc == 0 else load_chunk(sc)
            proj_chunk(sc, ins)
            attn_supertile(sc)
        for t in range(cfg["oproj_defer"]):
            out_proj(t)


def _host_prep(x, wq, wk, wv, wo):
    bf16 = ml_dtypes.bfloat16
    f8 = ml_dtypes.float8_e4m3
    fp8_qk = DEFAULT_CFG["fp8_qk"]
    xT = np.ascontiguousarray(np.asarray(x, np.float32)[0].T)
    xTb = xT.astype(bf16)
    if DEFAULT_CFG["x_contig"]:
        # device layout: rows = sc*128 + p, cols = kc*512 + s
        xTb = np.ascontiguousarray(
            xTb.reshape(16, 128, 8, 512).transpose(2, 1, 0, 3)
            .reshape(8 * 128, 16 * 512))

    inv_freq = 1.0 / (THETA ** (np.arange(0, HD, 2, np.float32) / HD))
    pos = np.arange(S, dtype=np.float32)
    freqs = pos[:, None] * inv_freq[None, :]
    emb = np.concatenate([freqs, freqs], axis=-1)      # [S, 128]
    cosT = np.cos(emb).T
    # sign-folded sin table: rows 0:64 negated (q' = q*cos + swap64(q)*sinT)
    sinT = np.sin(emb).T.copy()
    sinT[0:64, :] *= -1.0
    csT = np.ascontiguousarray(
        np.concatenate([cosT, sinT], axis=1)).astype(bf16)  # [128, 2S]

    kk = np.arange(128)[:, None]
    qq = np.arange(QSUP)[None, :]
    maskB = np.concatenate(
        [(qq >= kk + 128 * j) for j in range(4)], axis=1).astype(bf16)

    wq = np.asarray(wq, np.float32)
    wk = np.asarray(wk, np.float32)
    wv = np.asarray(wv, np.float32)
    wo = np.asarray(wo, np.float32)
    qk_t = f8 if fp8_qk else bf16

    def wprep(w, dtype):
        # [2048, C] -> device layout [128, 16*C] (rows p, cols kc*C + c)
        w = np.asarray(w, dtype)
        if not DEFAULT_CFG["w_contig"]:
            return np.ascontiguousarray(w)
        C = w.shape[1]
        return np.ascontiguousarray(
            w.reshape(16, 128, C).transpose(1, 0, 2).reshape(128, 16 * C))

    in_maps = []
    for c in range(NCORES):
        g = c // 2
        m = {
            "xT": xTb,
            "wq": wprep(wq[:, 256 * c:256 * c + 256], qk_t),
            "wk": wprep(wk[:, 128 * g:128 * g + 128], qk_t),
            "wv": wprep(wv[:, 128 * g:128 * g + 128], bf16),
            "wo": np.ascontiguousarray(wo[256 * c:256 * c + 256, :]).astype(bf16),
            "csT": csT,
            "maskB": maskB,
        }
        if fp8_qk:
            m["xT8"] = xT.astype(f8)
        in_maps.append(m)
    return in_maps


def get_nc():
    if "nc" not in _CACHE:
        _CACHE["nc"] = _build_nc()
    return _CACHE["nc"]


def kernel(x, wq, wk, wv, wo):
    from concourse.bass_utils import run_bass_kernel_spmd

    nc = get_nc()
    in_maps = _host_prep(x, wq, wk, wv, wo)
    res = run_bass_kernel_spmd(nc, in_maps, core_ids=list(range(NCORES)))
    _CACHE["last_results"] = res
    acc = res.results[0]["out"].astype(np.float32)
    for c in range(1, NCORES):
        acc = acc + res.results[c]["out"]
    return acc.reshape(1, S, DM)


# revision 67
# speedup vs baseline: 1.0023x; 1.0023x over previous
"""Trainium2 Bass kernel for a GQA attention block (dense_transformer).

Reference computation (per core c of 8, tensor-parallel over heads):
  q = x @ wq[:, 256c:256c+256]   -> 2 query heads of dim 128
  k = x @ wk[:, 128g:128g+128]   -> 1 kv head (g = c//2, shared by 2 cores)
  v = x @ wv[:, 128g:128g+128]
  RoPE on q, k; causal softmax attention; o = attn @ v
  out_partial = o @ wo[256c:256c+256, :]     (full [4096, 2048] partial sum)
Host sums the 8 partials.

Device layout: everything transposed ([dim, seq]) so matmuls need no
on-chip transposes of activations:
  QT/KT:  [128 d, 4096 s]  (from projections; RoPE applied on evacuation)
  scores^T[k, q] = KT_blk.T @ QT  (lhsT=KT block, rhs=QT columns)
  P^T = exp(scores^T) (ACT, scale=1/sqrt(128)); causal via 0/1 bf16 mask mul
  O^T += V_blk.T @ P^T            (lhsT=V block [s,d], rhs=P^T)
  row sums via ones-matmul into psum; normalization via K=1 broadcast matmul
  out[s, dm] = O^T_blk.T @ wo     (lhsT=O^T block, rhs=wo rows)

All matmuls are bf16 (fp8 q/k was tried and rejected: 4.5e-2 rel err vs the
2e-2 gate — with random-init weights softmax does NOT wash out logit noise).
Host pre-arranges x^T (and weights) so every DMA reads contiguous DRAM.
"""

import os
import numpy as np
import ml_dtypes

S = 4096
DM = 2048
HD = 128
NCORES = 8
QSUP = 512          # query supertile (free dim of scores^T psum)
NT = S // QSUP      # 8
NKB = S // 128      # 32 key blocks
SCALE = float(1.0 / np.sqrt(HD))
THETA = 10000.0

_CACHE = {}


def _tctile(tc, shape, dtype, name):
    return tc.nc.alloc_sbuf_tensor(name, list(shape), dtype).ap()



DEFAULT_CFG = dict(
    sums_on_dve=True,    # accumulate softmax row-sums on DVE (PE matmul-sums
                         # measured much slower on HW)
    out_f16=True,        # fp16 partial output (host sums in fp32)
    scps_bufs=3,         # psum bufs for scores (+shared r/out-proj/vt tiles)
    prps_bufs=2,         # psum bufs for projection accumulators
    pt_bufs=4,           # sbuf bufs for exp(P^T) tiles
    evac_alt=True,       # alternate out-proj psum evacuation DVE/ACT
    sums_gpsimd=False,   # offload head-1 row-sum adds to the idle Pool engine
    oacc_bufs=1,         # psum bufs per oacc head (2 overlaps t and t+1)
    rope_evac_dve=True,  # psum->sbuf rope copy on DVE instead of ACT
    diag_skip=True,      # skip all-zero columns [0,128j) of diagonal blocks
    r_dual_psum=False,   # ILLEGAL on HW: TT cannot read 2 PSUM operands
    xt_split=4,          # split the per-chunk x^T load into N DMAs
    sbp_bufs=2,          # bufs for the xt/cos/sin input pool
    fp8_qk=False,        # fp8 q/k proj: rel err 4.5e-2 > 2e-2 gate (dead)
    w_split=4,           # split weight DMAs into N kc-chunks (early start)
    out_pair=2,          # output subtiles per store DMA
    out_psum_dma=False,  # (unsupported: DMA cannot read PSUM)
    sums_split=False,    # even/odd kb sum accumulators (halves serial chain)
    evac_dve_tail=8,     # supertiles >= this evacuate out-proj on DVE only
    oproj_defer=0,       # defer first N supertiles' out-proj to the end
    skip_out_store=False,   # timing experiment: drop output stores
    skip_x_load=False,      # timing experiment: reread chunk 0 as all chunks
    fuse_attnv=False,    # ILLEGAL: fused psum out would span 2 banks
    x_contig=True,       # host pre-chunks xT so each chunk loads contiguously
    w_contig=True,       # host pre-transposes weights for contiguous loads
    ot_raw=True,         # evacuate oacc unnormalized (frees psum bank before
                         # the recip chain), normalize OT in place on DVE
    r_evac_dve=False,    # r_ps psum->sbuf copy on ACT (HW A/B: ~5us better
                         # than DVE, which is loaded by the raw OT copies)
    exp_pair_h=False,    # one exp instruction covering both heads' scores
                         # (2-bank psum tile; halves ACT instruction count)
    swap_act_dge=False,  # rope-swap DMAs on the ACT DGE ring: HW A/B was
                         # noise-contradictory; keep the validated SP ring
    pe_warmup=48,        # dummy PE transposes during the startup DMA wait:
                         # ramps the p-state so real matmuls start at full
                         # clock (PE runs 2x slower for ~3us after idle)
)


def _build_nc(loop_iters=1, cfg=None):
    import contextlib
    import concourse.mybir as mybir
    import concourse.tile as tile
    from concourse import bacc
    from concourse.masks import make_identity

    cfg = {**DEFAULT_CFG, **(cfg or {})}
    dt = mybir.dt
    f32 = dt.float32
    bf16 = dt.bfloat16
    f8 = dt.float8e4
    out_dt = f32 if cfg["out_psum_dma"] else (
        dt.float16 if cfg["out_f16"] else f32)

    nc = bacc.Bacc("TRN2", target_bir_lowering=False, debug=False, num_devices=NCORES)

    # x_contig: rows = sc*128 + p, cols = kc*512 + s (chunk-contiguous)
    xT_shape = [8 * 128, 16 * 512] if cfg["x_contig"] else [DM, S]
    xT_d = nc.dram_tensor("xT", xT_shape, bf16, kind="ExternalInput")
    wv_shape = [128, 16 * 128] if cfg["w_contig"] else [DM, 128]
    wv_d = nc.dram_tensor("wv", wv_shape, bf16, kind="ExternalInput")
    wo_d = nc.dram_tensor("wo", [256, DM], bf16, kind="ExternalInput")
    csT_d = nc.dram_tensor("csT", [HD, 2 * S], bf16, kind="ExternalInput")
    maskB_d = nc.dram_tensor("maskB", [128, 4 * QSUP], bf16, kind="ExternalInput")
    out_d = nc.dram_tensor("out", [S, DM], out_dt, kind="ExternalOutput")
    wq_shape = [128, 16 * 256] if cfg["w_contig"] else [DM, 256]
    wk_shape = [128, 16 * 128] if cfg["w_contig"] else [DM, 128]
    if cfg["fp8_qk"]:
        xT8_d = nc.dram_tensor("xT8", [DM, S], f8, kind="ExternalInput")
        wq_d = nc.dram_tensor("wq", wq_shape, f8, kind="ExternalInput")
        wk_d = nc.dram_tensor("wk", wk_shape, f8, kind="ExternalInput")
    else:
        xT8_d = None
        wq_d = nc.dram_tensor("wq", wq_shape, bf16, kind="ExternalInput")
        wk_d = nc.dram_tensor("wk", wk_shape, bf16, kind="ExternalInput")


    qk_dt = f8 if cfg["fp8_qk"] else bf16

    with tile.TileContext(nc) as tc:
        # ---- persistent sbuf tensors ----
        QT0 = _tctile(tc, [128, S], bf16, name="QT0")
        QT1 = _tctile(tc, [128, S], bf16, name="QT1")
        KT = _tctile(tc, [128, S], bf16, name="KT")
        VT = _tctile(tc, [128, S], bf16, name="VT")    # [d, s] pre-transpose
        V = _tctile(tc, [128, S], bf16, name="V")      # [s, d] blocks at cols 128*kb
        OT0 = _tctile(tc, [128, S], bf16, name="OT0")
        OT1 = _tctile(tc, [128, S], bf16, name="OT1")
        wq_sb = _tctile(tc, [128, 16 * 256], qk_dt, name="wq_sb")
        wk_sb = _tctile(tc, [128, 16 * 128], qk_dt, name="wk_sb")
        wv_sb = _tctile(tc, [128, 16 * 128], bf16, name="wv_sb")
        wo0_sb = _tctile(tc, [128, DM], bf16, name="wo0_sb")
        wo1_sb = _tctile(tc, [128, DM], bf16, name="wo1_sb")
        maskB = _tctile(tc, [128, 4 * QSUP], bf16, name="maskB_sb")
        ident = _tctile(tc, [128, 128], bf16, name="ident")
        onescols = _tctile(tc, [128, 3], bf16, name="onescols")
        ones1 = _tctile(tc, [33, 128], dt.float16, name="ones1")

        # constants
        make_identity(nc, ident[:, :])
        nc.gpsimd.memset(onescols[:, :], 0.0)
        nc.gpsimd.memset(onescols[:, 0:1], 1.0)
        nc.gpsimd.memset(onescols[:, 2:3], 1.0)
        nc.gpsimd.memset(ones1[0:1, :], 1.0)
        nc.gpsimd.memset(ones1[32:33, :], 1.0)

        env = dict(locals())
        loop_ctx = (tc.For_i(0, loop_iters, 1) if loop_iters > 1
                    else contextlib.nullcontext())
        with loop_ctx:
            _emit_body(nc, tc, mybir, cfg, env)

    nc.compile()
    return nc


def _emit_body(nc, tc, mybir, cfg, env):
    dt = mybir.dt
    f32 = dt.float32
    bf16 = dt.bfloat16
    f16 = dt.float16
    f8 = dt.float8e4
    out_dt = f32 if cfg["out_psum_dma"] else (f16 if cfg["out_f16"] else f32)
    AF = mybir.ActivationFunctionType
    PM = mybir.MatmulPerfMode
    xT = env["xT_d"].ap()
    out = env["out_d"].ap()
    csT_d = env["csT_d"]
    QT0, QT1, KT, VT, V, OT0, OT1 = (env[k] for k in
                                     ("QT0", "QT1", "KT", "VT", "V", "OT0", "OT1"))
    wq_sb, wk_sb, wv_sb, wo0_sb, wo1_sb = (env[k] for k in
                                           ("wq_sb", "wk_sb", "wv_sb",
                                            "wo0_sb", "wo1_sb"))
    maskB, ident, onescols, ones1 = (env[k] for k in
                                     ("maskB", "ident", "onescols", "ones1"))
    QTs = [QT0, QT1]
    OTs = [OT0, OT1]
    fp8_qk = cfg["fp8_qk"]

    def x_src(sc):
        """[128, 16, 512] source AP for chunk sc of x^T."""
        if cfg["x_contig"]:
            return (xT[128 * sc:128 * sc + 128, :]
                    .rearrange("p (kc s) -> p kc s", s=512))
        cs = slice(512 * sc, 512 * sc + 512)
        return xT.rearrange("(kc p) s -> p kc s", p=128)[:, :, cs]

    with tc.tile_pool(name="sbp", bufs=cfg["sbp_bufs"]) as sbp, \
         tc.tile_pool(name="prps", bufs=cfg["prps_bufs"], space="PSUM") as prps, \
         tc.tile_pool(name="scps", bufs=cfg["scps_bufs"], space="PSUM") as scps, \
         tc.tile_pool(name="oaps", bufs=cfg["oacc_bufs"], space="PSUM") as oaps, \
         tc.tile_pool(name="smps", bufs=1, space="PSUM") as smps, \
         tc.tile_pool(name="rop", bufs=3) as rop, \
         tc.tile_pool(name="ptpool", bufs=cfg["pt_bufs"]) as ptpool, \
         tc.tile_pool(name="nrm", bufs=2) as nrm, \
         tc.tile_pool(name="outsb", bufs=3) as outsb:

        def wps(shape, dtype, name):
            """Working psum tile: from prps in exp_pair_h mode (the scores
            ring then holds only 2-bank sc2 tiles), else from scps."""
            if cfg["exp_pair_h"]:
                return prps.tile(shape, dtype, tag="proj", name=name)
            return scps.tile(shape, dtype, tag="sc", name=name)

        def load_chunk(sc):
            """Issue input DMAs for s-chunk sc (512 wide); return tiles.

            fp8 x loads first (q/k proj consume it first); few big DMAs —
            each DMA instruction holds the shared HWDGE ring ~625ns, so
            instruction count dominates ring occupancy, not bytes.
            """
            if cfg["skip_x_load"]:
                sc = 0
            cs = slice(512 * sc, 512 * sc + 512)
            nsp = cfg["xt_split"]
            gk = 16 // nsp
            xt8 = None
            if fp8_qk:
                xt8 = sbp.tile([128, 16 * 512], f8, tag="xt8", name=f"xt8_{sc}")
                xt83 = xt8.rearrange("p (kc s) -> p kc s", s=512)
                src83 = (env["xT8_d"].ap()
                         .rearrange("(kc p) s -> p kc s", p=128)[:, :, cs])
                for g in range(nsp):
                    nc.sync.dma_start(xt83[:, gk * g:gk * (g + 1), :],
                                      src83[:, gk * g:gk * (g + 1), :])
            cs_t = sbp.tile([128, 1024], bf16, tag="cs", name=f"cs_{sc}")
            nc.sync.dma_start(
                cs_t.rearrange("p (i s) -> p i s", s=512),
                csT_d.ap().rearrange("p (i s) -> p i s", s=S)[:, :, cs])
            xt = sbp.tile([128, 16 * 512], bf16, tag="xt", name=f"xt_{sc}")
            xt3 = xt.rearrange("p (kc s) -> p kc s", s=512)
            src3 = x_src(sc)
            for g in range(nsp):
                nc.sync.dma_start(xt3[:, gk * g:gk * (g + 1), :],
                                  src3[:, gk * g:gk * (g + 1), :])
            return xt, xt8, cs_t[:, 0:512], cs_t[:, 512:1024]

        def load_preamble():
            """Weights + chunk-0 inputs, interleaved so the first q-proj
            matmul (wq kc-pair 0 + xt8 kc 0..1) unblocks ASAP."""
            nw = cfg["w_split"]
            gk = 16 // nw
            wq3 = wq_sb.rearrange("p (kc c) -> p kc c", c=256)
            wk3 = wk_sb.rearrange("p (kc c) -> p kc c", c=128)
            wv3 = wv_sb.rearrange("p (kc c) -> p kc c", c=128)
            if cfg["w_contig"]:
                wqs = env["wq_d"].ap().rearrange("p (kc c) -> p kc c", c=256)
                wks = env["wk_d"].ap().rearrange("p (kc c) -> p kc c", c=128)
                wvs = env["wv_d"].ap().rearrange("p (kc c) -> p kc c", c=128)
            else:
                wqs = env["wq_d"].ap().rearrange("(kc p) c -> p kc c", p=128)
                wks = env["wk_d"].ap().rearrange("(kc p) c -> p kc c", p=128)
                wvs = env["wv_d"].ap().rearrange("(kc p) c -> p kc c", p=128)
            cs = slice(0, 512)
            nsp = cfg["xt_split"]
            gx = 16 // nsp
            xt8 = None
            if fp8_qk:
                xt8 = sbp.tile([128, 16 * 512], f8, tag="xt8", name="xt8_0")
                xt83 = xt8.rearrange("p (kc s) -> p kc s", s=512)
                src83 = (env["xT8_d"].ap()
                         .rearrange("(kc p) s -> p kc s", p=128)[:, :, cs])
            xt = sbp.tile([128, 16 * 512], bf16, tag="xt", name="xt_0")
            xt3 = xt.rearrange("p (kc s) -> p kc s", s=512)
            src3 = x_src(0)
            # q/k weights + their x operand interleaved by kc quarter
            for g in range(max(nw, nsp)):
                if g < nw:
                    ks = slice(gk * g, gk * (g + 1))
                    nc.sync.dma_start(wq3[:, ks, :], wqs[:, ks, :])
                    nc.sync.dma_start(wk3[:, ks, :], wks[:, ks, :])
                if fp8_qk and g < nsp:
                    xs = slice(gx * g, gx * (g + 1))
                    nc.sync.dma_start(xt83[:, xs, :], src83[:, xs, :])
                if not fp8_qk and g < nsp:
                    xs = slice(gx * g, gx * (g + 1))
                    nc.sync.dma_start(xt3[:, xs, :], src3[:, xs, :])
            cs_t = sbp.tile([128, 1024], bf16, tag="cs", name="cs_0")
            nc.sync.dma_start(
                cs_t.rearrange("p (i s) -> p i s", s=512),
                csT_d.ap().rearrange("p (i s) -> p i s", s=S)[:, :, cs])
            # v weights (+ bf16 x when the q/k path is fp8)
            for g in range(max(nw, nsp)):
                if g < nw:
                    ks = slice(gk * g, gk * (g + 1))
                    nc.sync.dma_start(wv3[:, ks, :], wvs[:, ks, :])
                if fp8_qk and g < nsp:
                    xs = slice(gx * g, gx * (g + 1))
                    nc.sync.dma_start(xt3[:, xs, :], src3[:, xs, :])
            nc.sync.dma_start(maskB[:, :], env["maskB_d"].ap()[:, :])
            nc.sync.dma_start(wo0_sb[:, :], env["wo_d"].ap()[0:128, :])
            nc.sync.dma_start(wo1_sb[:, :], env["wo_d"].ap()[128:256, :])
            return xt, xt8, cs_t[:, 0:512], cs_t[:, 512:1024]

        def proj_chunk(sc, ins):
            """Projections + RoPE + V transpose for s-chunk sc (512 wide)."""
            cs = slice(512 * sc, 512 * sc + 512)
            xt, xt8, cos_t, sin_t = ins
            xt83 = (xt8.rearrange("p (kc s) -> p kc s", s=512)
                    if fp8_qk else None)

            def proj(w_sb, wstride, hofs, name):
                ps = prps.tile([128, 512], f32, tag="proj", name=name)
                for kc in range(16):
                    nc.tensor.matmul(
                        ps[:, :],
                        w_sb[:, wstride * kc + hofs:wstride * kc + hofs + 128],
                        xt[:, 512 * kc:512 * kc + 512],
                        start=(kc == 0), stop=(kc == 15))
                return ps

            def proj8(w_sb, wstride, hofs, name):
                ps = prps.tile([128, 512], f32, tag="proj", name=name)
                w3 = w_sb.rearrange("p (kc c) -> p kc c", c=wstride)
                for kp in range(8):
                    nc.tensor.matmul(
                        ps[:, :],
                        w3[:, 2 * kp:2 * kp + 2, hofs:hofs + 128],
                        xt83[:, 2 * kp:2 * kp + 2, :],
                        start=(kp == 0), stop=(kp == 7),
                        perf_mode=PM.DoubleRow)
                return ps

            pj = proj8 if fp8_qk else proj

            # shared raw/swap tiles for q0|q1|k: one swap DMA pair per chunk
            qraw3 = rop.tile([128, 1536], bf16, tag="qraw", name=f"qraw_{sc}")
            qsw3 = rop.tile([128, 1536], bf16, tag="qsw", name=f"qsw_{sc}")
            pss = [pj(wq_sb, 256, 0, f"psq0_{sc}"),
                   pj(wq_sb, 256, 128, f"psq1_{sc}"),
                   pj(wk_sb, 128, 0, f"psk_{sc}")]
            for h, ps in enumerate(pss):
                hs = slice(512 * h, 512 * h + 512)
                if cfg["rope_evac_dve"]:
                    nc.vector.tensor_copy(qraw3[:, hs], ps[:, :])
                else:
                    nc.scalar.copy(qraw3[:, hs], ps[:, :])
            dge = nc.scalar if cfg["swap_act_dge"] else nc.sync
            dge.dma_start(qsw3[0:64, :], qraw3[64:128, :])
            dge.dma_start(qsw3[64:128, :], qraw3[0:64, :])
            for h, dst in enumerate([QT0, QT1, KT]):
                hs = slice(512 * h, 512 * h + 512)
                m1 = rop.tile([128, 512], bf16, tag="m1", name=f"m1_{sc}_{h}")
                nc.vector.tensor_mul(m1[:, :], qraw3[:, hs], cos_t[:, :])
                nc.vector.tensor_mul(qsw3[:, hs], qsw3[:, hs], sin_t[:, :])
                nc.vector.tensor_add(dst[:, cs], m1[:, :], qsw3[:, hs])
            psv = proj(wv_sb, 128, 0, f"psv_{sc}")
            nc.vector.tensor_copy(VT[:, cs], psv[:, :])
            for kb in range(4 * sc, 4 * sc + 4):
                bs = slice(128 * kb, 128 * kb + 128)
                tp = wps([128, 128], bf16, name=f"vtp_{kb}")
                nc.tensor.transpose(tp[:, :], VT[:, bs], ident[:, :])
                nc.vector.tensor_copy(V[:, bs], tp[:, :])

        def attn_supertile(t):
            qs = slice(QSUP * t, QSUP * t + QSUP)
            nkb = 4 * t + 4
            ep = cfg["exp_pair_h"]
            fuse = cfg["fuse_attnv"]
            if fuse:
                oacc2 = oaps.tile([128, 2 * QSUP], f32, tag="oacc2",
                                  name=f"oacc2_{t}")
                oacc = [oacc2[:, 0:QSUP], oacc2[:, QSUP:2 * QSUP]]
            else:
                oacc = [oaps.tile([128, QSUP], f32, tag="oacc0",
                                  name=f"oacc0_{t}"),
                        oaps.tile([128, QSUP], f32, tag="oacc1",
                                  name=f"oacc1_{t}")]
            # rows {0, 32} of one psum bank (32-aligned for later matmul rhs)
            # with exp_pair_h the sums tile borrows a scores-ring slot at the
            # END of the supertile (early allocation would deadlock the ring)
            sums = (None if ep else
                    smps.tile([33, QSUP], f32, tag="sums", name=f"sums_{t}"))
            nsac = 2 if (cfg["sums_split"] and nkb > 4) else 1
            if cfg["sums_on_dve"]:
                sacc = [[nrm.tile([128, QSUP], bf16, tag=f"sacc{h}_{p}",
                                  name=f"sacc{h}_{p}_{t}")
                         for p in range(nsac)] for h in range(2)]
                sfirst = [[True] * nsac for _ in range(2)]
            prev_pt = [None, None]
            for kb in range(nkb):
                bs = slice(128 * kb, 128 * kb + 128)
                j = kb - 4 * t
                # columns [0, z) of a diagonal block are entirely masked
                z = 128 * j if (cfg["diag_skip"] and j > 0) else 0
                zq = slice(QSUP * t + z, QSUP * t + QSUP)
                pt2 = (ptpool.tile([128, 2 * QSUP], bf16, tag="pt",
                                   name=f"pt2_{t}_{kb}") if (fuse or ep)
                       else None)
                if ep:
                    sc2 = scps.tile([128, 2 * QSUP], f32, tag="sc",
                                    name=f"sc2_{t}_{kb}")
                    for h in range(2):
                        nc.tensor.matmul(
                            sc2[:, QSUP * h + z:QSUP * (h + 1)],
                            KT[:, bs], QTs[h][:, zq], start=True, stop=True)
                    if z:
                        nc.scalar.activation(
                            pt2.rearrange("p (h s) -> p h s", s=QSUP)[:, :, z:],
                            sc2.rearrange("p (h s) -> p h s", s=QSUP)[:, :, z:],
                            AF.Exp, scale=SCALE)
                    else:
                        nc.scalar.activation(pt2[:, :], sc2[:, :], AF.Exp,
                                             scale=SCALE)
                for h in range(2):
                    if ep:
                        pt = pt2[:, QSUP * h:QSUP * h + QSUP]
                    else:
                        sc_ps = scps.tile([128, QSUP], f32, tag="sc",
                                          name=f"sc_{t}_{kb}_{h}")
                        nc.tensor.matmul(sc_ps[:, z:], KT[:, bs],
                                         QTs[h][:, zq],
                                         start=True, stop=True)
                        pt = (pt2[:, QSUP * h:QSUP * h + QSUP] if fuse else
                              ptpool.tile([128, QSUP], bf16, tag="pt",
                                          name=f"pt_{t}_{kb}_{h}"))
                        nc.scalar.activation(pt[:, z:], sc_ps[:, z:], AF.Exp,
                                             scale=SCALE)
                    if 0 <= j:
                        w = 128 * (j + 1)   # cols >= w are fully unmasked
                        nc.vector.tensor_mul(
                            pt[:, z:w], pt[:, z:w],
                            maskB[:, QSUP * j + z:QSUP * j + w])
                    if not fuse:
                        nc.tensor.matmul(oacc[h][:, z:], V[:, bs], pt[:, z:],
                                         start=(kb == 0), stop=(kb == nkb - 1))
                    if cfg["sums_on_dve"]:
                        eng = (nc.gpsimd if (cfg["sums_gpsimd"] and h == 1)
                               else nc.vector)
                        p = kb % nsac
                        sa = sacc[h][p]
                        if sfirst[h][p]:
                            sfirst[h][p] = False
                            eng.tensor_copy(sa[:, z:], pt[:, z:])
                            if z:
                                nc.gpsimd.memset(sa[:, 0:z], 0.0)
                        else:
                            eng.tensor_add(sa[:, z:], sa[:, z:], pt[:, z:])
                    else:
                        nc.tensor.matmul(sums[32 * h:32 * h + 1, :],
                                         onescols[:, 0:1], pt[:, :],
                                         start=(kb == 0),
                                         stop=(kb == nkb - 1))
                if fuse:
                    rhs3 = pt2.rearrange("p (h s) -> p h s", s=QSUP)[:, :, z:]
                    out3 = (oacc2.rearrange("p (h s) -> p h s", s=QSUP)
                            [:, :, z:])
                    nc.tensor.matmul(out3, V[:, bs], rhs3,
                                     start=(kb == 0), stop=(kb == nkb - 1))
            if ep:
                sums = scps.tile([33, QSUP], f32, tag="sc", name=f"sums_{t}")
            if cfg["sums_on_dve"]:
                for h in range(2):
                    for p in range(nsac):
                        nc.tensor.matmul(sums[32 * h:32 * h + 1, :],
                                         onescols[:, 0:1], sacc[h][p][:, :],
                                         start=(p == 0), stop=(p == nsac - 1))
            # free the oacc psum bank ASAP: raw copy now, normalize in place
            if cfg["ot_raw"]:
                for h in range(2):
                    nc.vector.tensor_copy(OTs[h][:, qs], oacc[h][:, :])
            # normalize: rs = 1/sums (fp16), broadcast via K=1 fp16 matmul
            rs = nrm.tile([33, QSUP], f16, tag="rs", name=f"rs_{t}")
            with nc.allow_low_precision(reason="fp16 softmax normalizer"):
                nc.vector.reciprocal(rs[0:1, :], sums[0:1, :])
                nc.vector.reciprocal(rs[32:33, :], sums[32:33, :])
            for h in range(2):
                r_ps = wps([128, QSUP], f32, name=f"rps_{t}_{h}")
                nc.tensor.matmul(r_ps[:, :], ones1[32 * h:32 * h + 1, :],
                                 rs[32 * h:32 * h + 1, :],
                                 start=True, stop=True)
                r_sb = nrm.tile([128, QSUP], f16 if cfg["ot_raw"] else f32,
                                tag="rsb", name=f"rsb_{t}_{h}")
                if cfg["r_evac_dve"]:
                    nc.vector.tensor_copy(r_sb[:, :], r_ps[:, :])
                else:
                    nc.scalar.copy(r_sb[:, :], r_ps[:, :])
                if cfg["ot_raw"]:
                    nc.vector.tensor_mul(OTs[h][:, qs], OTs[h][:, qs],
                                         r_sb[:, :])
                else:
                    nc.vector.tensor_mul(OTs[h][:, qs], oacc[h][:, :],
                                         r_sb[:, :])
            if t < cfg["oproj_defer"]:
                return      # emitted at the end: PE filler for the exp tail
            out_proj(t)

        def out_proj(t):
            # out-projection for the 4 s-subtiles of supertile t;
            # out_pair subtiles share one sbuf tile -> one store DMA
            npair = 1 if t == NT - 1 else cfg["out_pair"]
            use_act = cfg["evac_alt"] and t < cfg["evac_dve_tail"]
            for u in range(4 // npair):
                ob = outsb.tile([128, npair * DM], out_dt, tag="ob",
                                name=f"ob_{t}_{u}")
                for two in range(npair):
                    st = 4 * t + npair * u + two
                    ss = slice(128 * st, 128 * st + 128)
                    for ncol in range(4):
                        o_ps = wps([128, 512], f32, name=f"op_{st}_{ncol}")
                        nc.tensor.matmul(o_ps[:, :], OT0[:, ss],
                                         wo0_sb[:, 512 * ncol:512 * ncol + 512],
                                         start=True, stop=False)
                        nc.tensor.matmul(o_ps[:, :], OT1[:, ss],
                                         wo1_sb[:, 512 * ncol:512 * ncol + 512],
                                         start=False, stop=True)
                        oc = slice(DM * two + 512 * ncol,
                                   DM * two + 512 * ncol + 512)
                        if use_act and (ncol % 2 == 1):
                            nc.scalar.copy(ob[:, oc], o_ps[:, :])
                        else:
                            nc.vector.tensor_copy(ob[:, oc], o_ps[:, :])
                rb = 128 * (4 * t + npair * u)
                if cfg["skip_out_store"]:
                    pass
                elif npair == 1:
                    nc.sync.dma_start(out[rb:rb + 128, :], ob[:, :])
                else:
                    nc.sync.dma_start(
                        out[rb:rb + 128 * npair, :]
                        .rearrange("(two p) c -> p two c", p=128),
                        ob.rearrange("p (two c) -> p two c", c=DM))

        pre = load_preamble()
        for wu in range(cfg["pe_warmup"]):
            wt = prps.tile([128, 128], bf16, tag="proj", name=f"wu_{wu}")
            nc.tensor.transpose(wt[:, :], ident[:, :], ident[:, :])
        for sc in range(8):
            ins = pre if sc == 0 else load_chunk(sc)
            proj_chunk(sc, ins)
            attn_supertile(sc)
        for t in range(cfg["oproj_defer"]):
            out_proj(t)


def _host_prep(x, wq, wk, wv, wo):
    bf16 = ml_dtypes.bfloat16
    f8 = ml_dtypes.float8_e4m3
    fp8_qk = DEFAULT_CFG["fp8_qk"]
    xT = np.ascontiguousarray(np.asarray(x, np.float32)[0].T)
    xTb = xT.astype(bf16)
    if DEFAULT_CFG["x_contig"]:
        # device layout: rows = sc*128 + p, cols = kc*512 + s
        xTb = np.ascontiguousarray(
            xTb.reshape(16, 128, 8, 512).transpose(2, 1, 0, 3)
            .reshape(8 * 128, 16 * 512))

    inv_freq = 1.0 / (THETA ** (np.arange(0, HD, 2, np.float32) / HD))
    pos = np.arange(S, dtype=np.float32)
    freqs = pos[:, None] * inv_freq[None, :]
    emb = np.concatenate([freqs, freqs], axis=-1)      # [S, 128]
    cosT = np.cos(emb).T
    # sign-folded sin table: rows 0:64 negated (q' = q*cos + swap64(q)*sinT)
    sinT = np.sin(emb).T.copy()
    sinT[0:64, :] *= -1.0
    csT = np.ascontiguousarray(
        np.concatenate([cosT, sinT], axis=1)).astype(bf16)  # [128, 2S]

    kk = np.arange(128)[:, None]
    qq = np.arange(QSUP)[None, :]
    maskB = np.concatenate(
        [(qq >= kk + 128 * j) for j in range(4)], axis=1).astype(bf16)

    wq = np.asarray(wq, np.float32)
    wk = np.asarray(wk, np.float32)
    wv = np.asarray(wv, np.float32)
    wo = np.asarray(wo, np.float32)
    qk_t = f8 if fp8_qk else bf16

    def wprep(w, dtype):
        # [2048, C] -> device layout [128, 16*C] (rows p, cols kc*C + c)
        w = np.asarray(w, dtype)
        if not DEFAULT_CFG["w_contig"]:
            return np.ascontiguousarray(w)
        C = w.shape[1]
        return np.ascontiguousarray(
            w.reshape(16, 128, C).transpose(1, 0, 2).reshape(128, 16 * C))

    in_maps = []
    for c in range(NCORES):
        g = c // 2
        m = {
            "xT": xTb,
            "wq": wprep(wq[:, 256 * c:256 * c + 256], qk_t),
            "wk": wprep(wk[:, 128 * g:128 * g + 128], qk_t),
            "wv": wprep(wv[:, 128 * g:128 * g + 128], bf16),
            "wo": np.ascontiguousarray(wo[256 * c:256 * c + 256, :]).astype(bf16),
            "csT": csT,
            "maskB": maskB,
        }
        if fp8_qk:
            m["xT8"] = xT.astype(f8)
        in_maps.append(m)
    return in_maps


def get_nc():
    if "nc" not in _CACHE:
        _CACHE["nc"] = _build_nc()
    return _CACHE["nc"]


def kernel(x, wq, wk, wv, wo):
    from concourse.bass_utils import run_bass_kernel_spmd

    nc = get_nc()
    in_maps = _host_prep(x, wq, wk, wv, wo)
    res = run_bass_kernel_spmd(nc, in_maps, core_ids=list(range(NCORES)))
    _CACHE["last_results"] = res
    acc = res.results[0]["out"].astype(np.float32)
    for c in range(1, NCORES):
        acc = acc + res.results[c]["out"]
    return acc.reshape(1, S, DM)
